# revision 19
# baseline (speedup 1.0000x reference)
"""Trainium2 Bass kernel for nn_Attention_86663850099018.

Math (per batch b, reference semantics):
    xn = x_b / ||x_b rows||                      # (N, E) row-normalized
    S  = xn @ xn.T                               # (N, N) cosine scores, symmetric, in [-1, 1]
    P  = softmax(S, axis=1)                      # row softmax over keys
    U  = P @ h_b                                 # (N, H)
    out = U / frob_norm(U over all batches)      # the reference's H* factor cancels

S is symmetric and bounded so softmax needs no max subtraction, and the
column block of E = exp(S) needed as the stationary operand of the second
matmul equals the row block computed naturally — no transpose of the score
matrix. Rows are relabeled p-major (row = p*16 + t).

Speed design (vs the fp16 serial baseline):
  - Phase A (scores) runs in fp8-e4m3 DoubleRow: the whole E=256
    contraction is one matmul at 0.5 cycles/row.  Rel-err ~2.3e-3
    (tolerance 2e-2).  Phase B (E @ h) stays fp16.
  - exp(S) on ACT is the phase-A wall (~36us); phase A blocks are
    interleaved with phase B column blocks on the PE queue so the PE
    crunches B matmuls while ACT exps the previous A block.
  - Phase B PSUM is drained raw (unnormalized U); row softmax scale and
    the global 1/frobnorm fold into ONE per-row scale at writeback.
    Row sums of exp(S) are per-slab DVE reduces spread through phase B.
  - Sum-of-squares per row via scalar_tensor_tensor accumulate
    (tensor_scalar instruction family; tensor_tensor_reduce faults the HW).
  - Global norm: one 4-byte AllGather + local combine, per core.

Sharding: data-parallel over batch B=8, one batch per NeuronCore.
"""

import numpy as np

N, B, E, H = 2048, 8, 256, 512
P = 128
NT = N // P      # 16 row tiles
EC = E // P      # 2 contraction chunks (DoubleRow pairs them)
SF = 512         # matmul free-dim chunk
FC = N // SF     # 4 score column chunks
NCORES = 8

WARMUP_CC = False     # dummy AllReduce at start to warm the CC stream
ALLREDUCE = False     # True: AllReduce(add); False: AllGather + local sum
DOUBLEROW = True      # fp8 DoubleRow phase A
_CACHE = {}


def _build():
    import concourse.mybir as mybir
    import concourse.tile as tile
    from concourse import bacc
    from concourse.masks import make_identity

    f32 = mybir.dt.float32
    f16 = mybir.dt.float16
    f8 = mybir.dt.float8e4
    AF = mybir.ActivationFunctionType
    ALU = mybir.AluOpType
    AX = mybir.AxisListType
    DR = mybir.MatmulPerfMode.DoubleRow

    nc = bacc.Bacc("TRN2", target_bir_lowering=False, debug=False, num_devices=NCORES)

    x_d = nc.dram_tensor("x", [N, E], f32, kind="ExternalInput").ap()
    h_d = nc.dram_tensor("h", [N, H], f32, kind="ExternalInput").ap()
    o_d = nc.dram_tensor("out", [N, H], f32, kind="ExternalOutput").ap()

    x_pt = x_d.rearrange("(p t) e -> p t e", t=NT)
    h_pt = h_d.rearrange("(p t) e -> p t e", t=NT)
    o_pt = o_d.rearrange("(p t) e -> p t e", t=NT)

    with tile.TileContext(nc) as tc:
        with (
            tc.tile_pool(name="const", bufs=1) as constp,
            tc.tile_pool(name="eexpp", bufs=1) as eexpp,
            tc.tile_pool(name="hp", bufs=1) as hp,
            tc.tile_pool(name="urawp", bufs=1) as urawp,
            tc.tile_pool(name="zp", bufs=1) as zp,
            tc.tile_pool(name="dramp", bufs=1, space="DRAM") as dramp,
        ):
            if WARMUP_CC:
                w_in = dramp.tile([1, 1], f32)
                w_out = dramp.tile([1, 1], f32)
                wz = zp.tile([1, 1], f32)
                nc.gpsimd.memset(wz[:], 0.0)
                nc.gpsimd.dma_start(w_in[:], wz[:])
                nc.gpsimd.collective_compute(
                    "AllReduce", ALU.add,
                    replica_groups=[list(range(NCORES))],
                    ins=[w_in.opt()], outs=[w_out.opt()],
                )

            ident = constp.tile([P, P], f16)
            make_identity(nc, ident[:])
            ones = constp.tile([P, 1], f32)
            nc.vector.memset(ones[:], 1.0)

            eexp = eexpp.tile([P, NT, N], f16)        # 64 KiB/partition
            h_sb = hp.tile([P, NT, H], f16)           # 16 KiB/partition
            uraw = urawp.tile([P, NT, H], f32)        # 32 KiB/partition
            xnt_ch = [
                constp.tile([P, EC, SF], f8, name=f"xnt{q}", tag=f"xnt{q}")
                for q in range(FC)
            ]                                          # 4 x 1 KiB/partition

            zpartT = zp.tile([P, FC, NT], f32)
            zsA = zp.tile([P, NT], f32)
            zsB = zp.tile([P, NT], f32)
            zsum = zp.tile([P, NT], f32)
            zinv = zp.tile([P, NT], f32)
            ssqraw = zp.tile([P, NT], f32)
            rs = zp.tile([P, NT], f32)
            t1 = zp.tile([P, NT], f32)
            t2 = zp.tile([P, NT], f32)
            sscol = zp.tile([P, 1], f32)
            sq_scr = zp.tile([P, SF], f32)

            # ---------------- phase 0: load, normalize, transpose -------
            XCH = 8                                    # chunks of 2 row tiles
            with (
                tc.tile_pool(name="xtrans", bufs=1) as xtp,
                tc.tile_pool(name="ph0", bufs=3) as ph0,
                tc.tile_pool(name="psT", bufs=2, space="PSUM") as psT,
            ):
                x_all = xtp.tile([P, NT, E], f32)     # 16 KiB/partition
                htmp = xtp.tile([P, NT, H], f32)      # 32 KiB/partition
                ssq_all = xtp.tile([P, NT], f32)
                lnssq = xtp.tile([P, NT], f32)
                invn = xtp.tile([P, NT], f32)

                xengs = [nc.sync, nc.scalar, nc.gpsimd]
                for ch in range(XCH):
                    t0 = 2 * ch
                    xengs[ch % 3].dma_start(
                        x_all[:, t0 : t0 + 2, :], x_pt[:, t0 : t0 + 2, :]
                    )
                    scr = ph0.tile([P, 2, E], f32, tag="scr")
                    nc.scalar.activation(
                        scr[:], x_all[:, t0 : t0 + 2, :], AF.Square
                    )
                    nc.vector.tensor_reduce(
                        ssq_all[:, t0 : t0 + 2], scr[:], axis=AX.X, op=ALU.add
                    )
                    nc.scalar.activation(
                        lnssq[:, t0 : t0 + 2], ssq_all[:, t0 : t0 + 2], AF.Sqrt
                    )
                    nc.vector.reciprocal(
                        invn[:, t0 : t0 + 2], lnssq[:, t0 : t0 + 2]
                    )
                    for t in (t0, t0 + 1):
                        xn = ph0.tile([P, E], f16, tag="xn")
                        # balance the normalize multiplies across ACT / DVE
                        if t % 2 == 0:
                            nc.scalar.activation(
                                xn[:], x_all[:, t, :], AF.Copy,
                                scale=invn[:, t : t + 1],
                            )
                        else:
                            nc.vector.tensor_scalar_mul(
                                xn[:], x_all[:, t, :], invn[:, t : t + 1]
                            )
                        pt = psT.tile([P, EC, P], f16, tag="pt")
                        for c in range(EC):
                            nc.tensor.transpose(
                                pt[:, c, :], xn[:, c * P : (c + 1) * P], ident[:]
                            )
                        # PSUM->SBUF copy quantizes to fp8
                        nc.vector.tensor_copy(
                            xnt_ch[t // 4][:, :, (t % 4) * P : (t % 4 + 1) * P],
                            pt[:],
                        )

                hengs = [nc.gpsimd, nc.gpsimd, nc.sync, nc.scalar]
                for k in range(4):
                    hengs[k].dma_start(
                        htmp[:, 4 * k : 4 * k + 4, :], h_pt[:, 4 * k : 4 * k + 4, :]
                    )
                for k in range(4):
                    nc.vector.tensor_copy(
                        h_sb[:, 4 * k : 4 * k + 4, :], htmp[:, 4 * k : 4 * k + 4, :]
                    )

            # ---------------- phases A+B interleaved ---------------------
            with (
                tc.tile_pool(name="psA", bufs=2, space="PSUM") as psA,
                tc.tile_pool(name="psB", bufs=3, space="PSUM") as psB,
                tc.tile_pool(name="psS", bufs=1, space="PSUM") as psS,
            ):
                def a_block(jc):
                    # 8 matmul pairs + exp straight from PSUM (ACT paces
                    # PSUM recycling; B blocks interleave into the gaps)
                    for ip in range(NT // 2):
                        ps = psA.tile([P, 2, SF], f32, tag="psA")
                        for u in range(2):
                            i = 2 * ip + u
                            if DOUBLEROW:
                                nc.tensor.matmul(
                                    ps[:, u, :],
                                    xnt_ch[i // 4][:, :, (i % 4) * P : (i % 4 + 1) * P],
                                    xnt_ch[jc][:],
                                    start=True, stop=True, perf_mode=DR,
                                )
                            else:
                                for c in range(EC):
                                    nc.tensor.matmul(
                                        ps[:, u, :],
                                        xnt_ch[i // 4][:, c, (i % 4) * P : (i % 4 + 1) * P],
                                        xnt_ch[jc][:, c, :],
                                        start=(c == 0), stop=(c == EC - 1),
                                    )
                        nc.scalar.activation(
                            eexp[:, 2 * ip : 2 * ip + 2, jc * SF : (jc + 1) * SF],
                            ps[:], AF.Exp,
                        )

                def b_block(jcb):
                    # 4 output column blocks; DVE drains U raw, does the
                    # per-slab softmax row sums and the row ssq accumulate
                    for j in range(4 * jcb, 4 * jcb + 4):
                        ps = psB.tile([P, H], f32, tag="psB")
                        for i in range(NT):
                            nc.tensor.matmul(
                                ps[:],
                                eexp[:, i, j * P : (j + 1) * P],
                                h_sb[:, i, :],
                                start=(i == 0), stop=(i == NT - 1),
                            )
                        nc.vector.tensor_copy(uraw[:, j, :], ps[:])
                        nc.vector.scalar_tensor_tensor(
                            out=sq_scr[:],
                            in0=uraw[:, j, :],
                            scalar=1.0,
                            in1=uraw[:, j, :],
                            op0=ALU.mult,
                            op1=ALU.mult,
                            accum_out=ssqraw[:, j : j + 1],
                        )
                        # exp row-sum slabs k = jc*8 + ip, two per j
                        for k in (2 * j, 2 * j + 1):
                            jc, ip = k // 8, k % 8
                            nc.vector.tensor_reduce(
                                zpartT[:, jc, 2 * ip : 2 * ip + 2],
                                eexp[:, 2 * ip : 2 * ip + 2, jc * SF : (jc + 1) * SF],
                                axis=AX.X,
                                op=ALU.add,
                            )

                a_block(0)
                a_block(1)
                b_block(0)
                a_block(2)
                b_block(1)
                a_block(3)
                # preload the sqrt/copy ACT table before the tail needs it
                sqpre = zp.tile([1, 1], f32)
                nc.scalar.activation(sqpre[:], ones[:1, :1], AF.Sqrt)
                b_block(2)
                b_block(3)

                # ---------------- tail: global norm + writeback ----------
                nc.vector.scalar_tensor_tensor(
                    out=zsA[:], in0=zpartT[:, 0, :], scalar=0.0,
                    in1=zpartT[:, 1, :], op0=ALU.add, op1=ALU.add,
                )
                nc.vector.scalar_tensor_tensor(
                    out=zsB[:], in0=zpartT[:, 2, :], scalar=0.0,
                    in1=zpartT[:, 3, :], op0=ALU.add, op1=ALU.add,
                )
                nc.vector.scalar_tensor_tensor(
                    out=zsum[:], in0=zsA[:], scalar=0.0,
                    in1=zsB[:], op0=ALU.add, op1=ALU.add,
                )
                nc.vector.reciprocal(zinv[:], zsum[:])
                nc.vector.scalar_tensor_tensor(
                    out=t1[:], in0=ssqraw[:], scalar=1.0,
                    in1=zinv[:], op0=ALU.mult, op1=ALU.mult,
                )
                nc.vector.scalar_tensor_tensor(
                    out=t2[:], in0=t1[:], scalar=1.0,
                    in1=zinv[:], op0=ALU.mult, op1=ALU.mult,
                    accum_out=sscol[:],
                )

                ps1 = psS.tile([1, 1], f32, tag="ps1")
                nc.tensor.matmul(ps1[:], ones[:], sscol[:], start=True, stop=True)
                ss11 = zp.tile([1, 1], f32)
                nc.scalar.copy(ss11[:], ps1[:])

                cc_in = dramp.tile([1, 1], f32)
                gg = zp.tile([1, 1], f32)
                nc.gpsimd.dma_start(cc_in[:], ss11[:])
                if ALLREDUCE:
                    cc_out = dramp.tile([1, 1], f32)
                    nc.gpsimd.collective_compute(
                        "AllReduce", ALU.add,
                        replica_groups=[list(range(NCORES))],
                        ins=[cc_in.opt()], outs=[cc_out.opt()],
                    )
                    nc.sync.dma_start(gg[:], cc_out[:])
                else:
                    cc_out = dramp.tile([NCORES, 1], f32)
                    nc.gpsimd.collective_compute(
                        "AllGather", ALU.bypass,
                        replica_groups=[list(range(NCORES))],
                        ins=[cc_in.opt()], outs=[cc_out.opt()],
                    )
                    agg = zp.tile([NCORES, 1], f32)
                    nc.sync.dma_start(agg[:], cc_out[:])
                    ps2 = psS.tile([1, 1], f32, tag="ps1")
                    nc.tensor.matmul(
                        ps2[:], ones[:NCORES, :], agg[:], start=True, stop=True
                    )
                    nc.scalar.copy(gg[:], ps2[:])

                lnt = zp.tile([1, 1], f32)
                gsc = zp.tile([1, 1], f32)
                nc.scalar.activation(lnt[:], gg[:], AF.Sqrt)
                nc.vector.reciprocal(gsc[:], lnt[:])
                gbc = zp.tile([P, 1], f32)
                nc.gpsimd.partition_broadcast(gbc[:], gsc[:])
                # fold row softmax scale and global norm into one scale
                nc.vector.tensor_scalar_mul(rs[:], zinv[:], gbc[:])

                wengs = [nc.sync, nc.scalar, nc.gpsimd, nc.sync]
                for j in range(NT):
                    if j % 2 == 0:
                        nc.vector.tensor_scalar_mul(
                            uraw[:, j, :], uraw[:, j, :], rs[:, j : j + 1]
                        )
                    else:
                        nc.scalar.activation(
                            uraw[:, j, :], uraw[:, j, :], AF.Copy,
                            scale=rs[:, j : j + 1],
                        )
                        wengs[(j // 2) % 4].dma_start(
                            o_pt[:, j - 1 : j + 1, :], uraw[:, j - 1 : j + 1, :]
                        )

    nc.compile()
    return nc


def _get_nc():
    if "nc" not in _CACHE:
        _CACHE["nc"] = _build()
    return _CACHE["nc"]


def _in_maps(x, h):
    return [
        {
            "x": np.ascontiguousarray(x[:, c, :]),
            "h": np.ascontiguousarray(h[:, c, :]),
        }
        for c in range(NCORES)
    ]


def kernel(x, h):
    from concourse.bass_utils import run_bass_kernel_spmd

    x = np.asarray(x, dtype=np.float32)
    h = np.asarray(h, dtype=np.float32)
    assert x.shape == (N, B, E) and h.shape == (N, B, H)

    nc = _get_nc()
    res = run_bass_kernel_spmd(nc, _in_maps(x, h), core_ids=list(range(NCORES)))
    out = np.empty((N, B, H), dtype=np.float32)
    for c in range(NCORES):
        out[:, c, :] = res.results[c]["out"]
    return out


# Exposed for test.py: run once with tracing to get hardware exec time.
def run_traced(x, h):
    import os
    import shutil

    from concourse.bass_utils import run_bass_kernel_spmd

    x = np.asarray(x, dtype=np.float32)
    h = np.asarray(h, dtype=np.float32)
    nc = _get_nc()
    tdir = "/root/problem/trace_out"
    shutil.rmtree(tdir, ignore_errors=True)
    os.makedirs(tdir, exist_ok=True)
    res = run_bass_kernel_spmd(
        nc, _in_maps(x, h), core_ids=list(range(NCORES)), trace=True, tmpdir=tdir
    )
    out = np.empty((N, B, H), dtype=np.float32)
    for c in range(NCORES):
        out[:, c, :] = res.results[c]["out"]
    return out, res


# revision 20
# speedup vs baseline: 1.1090x; 1.1090x over previous
"""Trainium2 Bass kernel for nn_Attention_86663850099018.

Math (per batch b, reference semantics):
    xn = x_b / ||x_b rows||                      # (N, E) row-normalized
    S  = xn @ xn.T                               # (N, N) cosine scores, symmetric, in [-1, 1]
    P  = softmax(S, axis=1)                      # row softmax over keys
    U  = P @ h_b                                 # (N, H)
    out = U / frob_norm(U over all batches)      # the reference's H* factor cancels

S is symmetric and bounded so softmax needs no max subtraction, and the
column block of E = exp(S) needed as the stationary operand of the second
matmul equals the row block computed naturally — no transpose of the score
matrix. Rows are relabeled p-major (row = p*16 + t).

Speed design (vs the fp16 serial baseline):
  - Phase A (scores) runs in fp8-e4m3 DoubleRow: the whole E=256
    contraction is one matmul at 0.5 cycles/row.  Rel-err ~2.3e-3
    (tolerance 2e-2).  Phase B (E @ h) stays fp16.
  - exp(S) on ACT is the phase-A wall (~36us); phase A blocks are
    interleaved with phase B column blocks on the PE queue so the PE
    crunches B matmuls while ACT exps the previous A block.
  - Phase B PSUM is drained raw (unnormalized U); row softmax scale and
    the global 1/frobnorm fold into ONE per-row scale at writeback.
    Row sums of exp(S) are per-slab DVE reduces spread through phase B.
  - Sum-of-squares per row via scalar_tensor_tensor accumulate
    (tensor_scalar instruction family; tensor_tensor_reduce faults the HW).
  - Global norm: one 4-byte AllGather + local combine, per core.

Sharding: data-parallel over batch B=8, one batch per NeuronCore.
"""

import numpy as np

N, B, E, H = 2048, 8, 256, 512
P = 128
NT = N // P      # 16 row tiles
EC = E // P      # 2 contraction chunks (DoubleRow pairs them)
SF = 512         # matmul free-dim chunk
FC = N // SF     # 4 score column chunks
NCORES = 8

WARMUP_CC = True      # dummy AllReduce at start to warm the CC stream
ALLREDUCE = False     # True: AllReduce(add); False: AllGather + local sum
DOUBLEROW = True      # fp8 DoubleRow phase A
_CACHE = {}


def _build():
    import concourse.mybir as mybir
    import concourse.tile as tile
    from concourse import bacc
    from concourse.masks import make_identity

    f32 = mybir.dt.float32
    f16 = mybir.dt.float16
    f8 = mybir.dt.float8e4
    AF = mybir.ActivationFunctionType
    ALU = mybir.AluOpType
    AX = mybir.AxisListType
    DR = mybir.MatmulPerfMode.DoubleRow

    nc = bacc.Bacc("TRN2", target_bir_lowering=False, debug=False, num_devices=NCORES)

    x_d = nc.dram_tensor("x", [N, E], f32, kind="ExternalInput").ap()
    h_d = nc.dram_tensor("h", [N, H], f32, kind="ExternalInput").ap()
    o_d = nc.dram_tensor("out", [N, H], f16, kind="ExternalOutput").ap()

    x_pt = x_d.rearrange("(p t) e -> p t e", t=NT)
    h_pt = h_d.rearrange("(p t) e -> p t e", t=NT)
    o_pt = o_d.rearrange("(p t) e -> p t e", t=NT)

    with tile.TileContext(nc) as tc:
        with (
            tc.tile_pool(name="const", bufs=1) as constp,
            tc.tile_pool(name="eexpp", bufs=1) as eexpp,
            tc.tile_pool(name="hp", bufs=1) as hp,
            tc.tile_pool(name="urawp", bufs=1) as urawp,
            tc.tile_pool(name="zp", bufs=1) as zp,
            tc.tile_pool(name="dramp", bufs=1, space="DRAM") as dramp,
        ):
            if WARMUP_CC:
                w_in = dramp.tile([1, 1], f32)
                w_out = dramp.tile([1, 1], f32)
                wz = zp.tile([1, 1], f32)
                nc.gpsimd.memset(wz[:], 0.0)
                nc.gpsimd.dma_start(w_in[:], wz[:])
                nc.gpsimd.collective_compute(
                    "AllReduce", ALU.add,
                    replica_groups=[list(range(NCORES))],
                    ins=[w_in.opt()], outs=[w_out.opt()],
                )

            ident = constp.tile([P, P], f16)
            make_identity(nc, ident[:])
            ones = constp.tile([P, 1], f32)
            nc.vector.memset(ones[:], 1.0)

            eexp = eexpp.tile([P, NT, N], f16)        # 64 KiB/partition
            h_sb = hp.tile([P, NT, H], f16)           # 16 KiB/partition
            uraw = urawp.tile([P, NT, H], f32)        # 32 KiB/partition
            of16 = urawp.tile([P, NT, H], f16)        # 16 KiB/partition
            xnt_ch = [
                constp.tile([P, EC, SF], f8, name=f"xnt{q}", tag=f"xnt{q}")
                for q in range(FC)
            ]                                          # 4 x 1 KiB/partition

            zpartT = zp.tile([P, FC, NT], f16)
            zsA = zp.tile([P, NT], f32)
            zsB = zp.tile([P, NT], f32)
            zsum = zp.tile([P, NT], f32)
            zinv = zp.tile([P, NT], f32)
            ssqraw = zp.tile([P, NT], f32)
            rs = zp.tile([P, NT], f32)
            t1 = zp.tile([P, NT], f32)
            t2 = zp.tile([P, NT], f32)
            sscol = zp.tile([P, 1], f32)
            sq_scr = zp.tile([P, SF], f32)

            # ---------------- phase 0: load, normalize, transpose -------
            XCH = 8                                    # chunks of 2 row tiles
            with (
                tc.tile_pool(name="xtrans", bufs=1) as xtp,
                tc.tile_pool(name="ph0", bufs=3) as ph0,
                tc.tile_pool(name="psT", bufs=2, space="PSUM") as psT,
            ):
                x_all = xtp.tile([P, NT, E], f32)     # 16 KiB/partition
                htmp = xtp.tile([P, NT, H], f32)      # 32 KiB/partition
                ssq_all = xtp.tile([P, NT], f32)
                lnssq = xtp.tile([P, NT], f32)
                invn = xtp.tile([P, NT], f32)

                xengs = [nc.sync, nc.scalar]
                for ch in range(XCH):
                    t0 = 2 * ch
                    xengs[ch % 2].dma_start(
                        x_all[:, t0 : t0 + 2, :], x_pt[:, t0 : t0 + 2, :]
                    )
                    scr = ph0.tile([P, 2, E], f32, tag="scr")
                    nc.scalar.activation(
                        scr[:], x_all[:, t0 : t0 + 2, :], AF.Square
                    )
                    nc.vector.tensor_reduce(
                        ssq_all[:, t0 : t0 + 2], scr[:], axis=AX.X, op=ALU.add
                    )
                    nc.scalar.activation(
                        lnssq[:, t0 : t0 + 2], ssq_all[:, t0 : t0 + 2], AF.Sqrt
                    )
                    nc.vector.reciprocal(
                        invn[:, t0 : t0 + 2], lnssq[:, t0 : t0 + 2]
                    )
                    for t in (t0, t0 + 1):
                        xn = ph0.tile([P, E], f16, tag="xn")
                        # balance the normalize multiplies across ACT / DVE
                        if t % 2 == 0:
                            nc.scalar.activation(
                                xn[:], x_all[:, t, :], AF.Copy,
                                scale=invn[:, t : t + 1],
                            )
                        else:
                            nc.vector.tensor_scalar_mul(
                                xn[:], x_all[:, t, :], invn[:, t : t + 1]
                            )
                        pt = psT.tile([P, EC, P], f16, tag="pt")
                        for c in range(EC):
                            nc.tensor.transpose(
                                pt[:, c, :], xn[:, c * P : (c + 1) * P], ident[:]
                            )
                        # PSUM->SBUF copy quantizes to fp8
                        nc.vector.tensor_copy(
                            xnt_ch[t // 4][:, :, (t % 4) * P : (t % 4 + 1) * P],
                            pt[:],
                        )

                # h rides the otherwise-idle gpsimd queue end-to-end so the
                # fp16 casts can never block DVE phase-0 work (the Tile
                # scheduler orders by its own sim-time readiness)
                for k in range(4):
                    nc.gpsimd.dma_start(
                        htmp[:, 4 * k : 4 * k + 4, :], h_pt[:, 4 * k : 4 * k + 4, :]
                    )
                    nc.gpsimd.tensor_copy(
                        h_sb[:, 4 * k : 4 * k + 4, :], htmp[:, 4 * k : 4 * k + 4, :]
                    )

            # ---------------- phases A+B interleaved ---------------------
            with (
                tc.tile_pool(name="psA", bufs=2, space="PSUM") as psA,
                tc.tile_pool(name="psB", bufs=3, space="PSUM") as psB,
                tc.tile_pool(name="psS", bufs=1, space="PSUM") as psS,
            ):
                def a_block(jc):
                    # 8 matmul pairs + exp straight from PSUM (ACT paces
                    # PSUM recycling; B blocks interleave into the gaps)
                    for ip in range(NT // 2):
                        ps = psA.tile([P, 2, SF], f32, tag="psA")
                        for u in range(2):
                            i = 2 * ip + u
                            if DOUBLEROW:
                                nc.tensor.matmul(
                                    ps[:, u, :],
                                    xnt_ch[i // 4][:, :, (i % 4) * P : (i % 4 + 1) * P],
                                    xnt_ch[jc][:],
                                    start=True, stop=True, perf_mode=DR,
                                )
                            else:
                                for c in range(EC):
                                    nc.tensor.matmul(
                                        ps[:, u, :],
                                        xnt_ch[i // 4][:, c, (i % 4) * P : (i % 4 + 1) * P],
                                        xnt_ch[jc][:, c, :],
                                        start=(c == 0), stop=(c == EC - 1),
                                    )
                        nc.scalar.activation(
                            eexp[:, 2 * ip : 2 * ip + 2, jc * SF : (jc + 1) * SF],
                            ps[:], AF.Exp,
                        )

                def b_block(jcb):
                    # 4 output column blocks; DVE drains U raw, does the
                    # per-slab softmax row sums and the row ssq accumulate
                    for j in range(4 * jcb, 4 * jcb + 4):
                        ps = psB.tile([P, H], f32, tag="psB")
                        for i in range(NT):
                            nc.tensor.matmul(
                                ps[:],
                                eexp[:, i, j * P : (j + 1) * P],
                                h_sb[:, i, :],
                                start=(i == 0), stop=(i == NT - 1),
                            )
                        nc.vector.tensor_copy(uraw[:, j, :], ps[:])
                        nc.vector.scalar_tensor_tensor(
                            out=sq_scr[:],
                            in0=uraw[:, j, :],
                            scalar=1.0,
                            in1=uraw[:, j, :],
                            op0=ALU.mult,
                            op1=ALU.mult,
                            accum_out=ssqraw[:, j : j + 1],
                        )
                        # exp row-sum slabs k = jc*8 + ip, two per j
                        for k in (2 * j, 2 * j + 1):
                            jc, ip = k // 8, k % 8
                            with nc.allow_low_precision(reason="zslab fp16, rel 7e-4"):
                                nc.vector.tensor_reduce(
                                    zpartT[:, jc, 2 * ip : 2 * ip + 2],
                                    eexp[:, 2 * ip : 2 * ip + 2, jc * SF : (jc + 1) * SF],
                                    axis=AX.X,
                                    op=ALU.add,
                                )

                a_block(0)
                a_block(1)
                b_block(0)
                a_block(2)
                b_block(1)
                a_block(3)
                # preload the sqrt/copy ACT table before the tail needs it
                sqpre = zp.tile([1, 1], f32)
                nc.scalar.activation(sqpre[:], ones[:1, :1], AF.Sqrt)
                b_block(2)
                b_block(3)

                # ---------------- tail: global norm + writeback ----------
                nc.vector.scalar_tensor_tensor(
                    out=zsA[:], in0=zpartT[:, 0, :], scalar=0.0,
                    in1=zpartT[:, 1, :], op0=ALU.add, op1=ALU.add,
                )
                nc.vector.scalar_tensor_tensor(
                    out=zsB[:], in0=zpartT[:, 2, :], scalar=0.0,
                    in1=zpartT[:, 3, :], op0=ALU.add, op1=ALU.add,
                )
                nc.vector.scalar_tensor_tensor(
                    out=zsum[:], in0=zsA[:], scalar=0.0,
                    in1=zsB[:], op0=ALU.add, op1=ALU.add,
                )
                nc.vector.reciprocal(zinv[:], zsum[:])
                nc.vector.scalar_tensor_tensor(
                    out=t1[:], in0=ssqraw[:], scalar=1.0,
                    in1=zinv[:], op0=ALU.mult, op1=ALU.mult,
                )
                nc.vector.scalar_tensor_tensor(
                    out=t2[:], in0=t1[:], scalar=1.0,
                    in1=zinv[:], op0=ALU.mult, op1=ALU.mult,
                    accum_out=sscol[:],
                )

                ps1 = psS.tile([1, 1], f32, tag="ps1")
                nc.tensor.matmul(ps1[:], ones[:], sscol[:], start=True, stop=True)
                ss11 = zp.tile([1, 1], f32)
                nc.scalar.copy(ss11[:], ps1[:])

                cc_in = dramp.tile([1, 1], f32)
                gg = zp.tile([1, 1], f32)
                nc.gpsimd.dma_start(cc_in[:], ss11[:])
                if ALLREDUCE:
                    cc_out = dramp.tile([1, 1], f32)
                    nc.gpsimd.collective_compute(
                        "AllReduce", ALU.add,
                        replica_groups=[list(range(NCORES))],
                        ins=[cc_in.opt()], outs=[cc_out.opt()],
                    )
                    nc.sync.dma_start(gg[:], cc_out[:])
                else:
                    cc_out = dramp.tile([NCORES, 1], f32)
                    nc.gpsimd.collective_compute(
                        "AllGather", ALU.bypass,
                        replica_groups=[list(range(NCORES))],
                        ins=[cc_in.opt()], outs=[cc_out.opt()],
                    )
                    agg = zp.tile([NCORES, 1], f32)
                    nc.sync.dma_start(agg[:], cc_out[:])
                    ps2 = psS.tile([1, 1], f32, tag="ps1")
                    nc.tensor.matmul(
                        ps2[:], ones[:NCORES, :], agg[:], start=True, stop=True
                    )
                    nc.scalar.copy(gg[:], ps2[:])

                lnt = zp.tile([1, 1], f32)
                gsc = zp.tile([1, 1], f32)
                nc.scalar.activation(lnt[:], gg[:], AF.Sqrt)
                nc.vector.reciprocal(gsc[:], lnt[:])
                gbc = zp.tile([P, 1], f32)
                nc.gpsimd.partition_broadcast(gbc[:], gsc[:])
                # fold row softmax scale and global norm into one scale
                nc.vector.tensor_scalar_mul(rs[:], zinv[:], gbc[:])

                wengs = [nc.sync, nc.gpsimd, nc.sync, nc.gpsimd]
                for j in range(NT):
                    if j % 2 == 0:
                        nc.vector.tensor_scalar_mul(
                            of16[:, j, :], uraw[:, j, :], rs[:, j : j + 1]
                        )
                    else:
                        nc.scalar.activation(
                            of16[:, j, :], uraw[:, j, :], AF.Copy,
                            scale=rs[:, j : j + 1],
                        )
                        wengs[(j // 2) % 4].dma_start(
                            o_pt[:, j - 1 : j + 1, :], of16[:, j - 1 : j + 1, :]
                        )

    nc.compile()
    return nc


def _get_nc():
    if "nc" not in _CACHE:
        _CACHE["nc"] = _build()
    return _CACHE["nc"]


def _in_maps(x, h):
    return [
        {
            "x": np.ascontiguousarray(x[:, c, :]),
            "h": np.ascontiguousarray(h[:, c, :]),
        }
        for c in range(NCORES)
    ]


def kernel(x, h):
    from concourse.bass_utils import run_bass_kernel_spmd

    x = np.asarray(x, dtype=np.float32)
    h = np.asarray(h, dtype=np.float32)
    assert x.shape == (N, B, E) and h.shape == (N, B, H)

    nc = _get_nc()
    res = run_bass_kernel_spmd(nc, _in_maps(x, h), core_ids=list(range(NCORES)))
    out = np.empty((N, B, H), dtype=np.float32)
    for c in range(NCORES):
        out[:, c, :] = res.results[c]["out"].astype(np.float32)
    return out


# Exposed for test.py: run once with tracing to get hardware exec time.
def run_traced(x, h):
    import os
    import shutil

    from concourse.bass_utils import run_bass_kernel_spmd

    x = np.asarray(x, dtype=np.float32)
    h = np.asarray(h, dtype=np.float32)
    nc = _get_nc()
    tdir = "/root/problem/trace_out"
    shutil.rmtree(tdir, ignore_errors=True)
    os.makedirs(tdir, exist_ok=True)
    res = run_bass_kernel_spmd(
        nc, _in_maps(x, h), core_ids=list(range(NCORES)), trace=True, tmpdir=tdir
    )
    out = np.empty((N, B, H), dtype=np.float32)
    for c in range(NCORES):
        out[:, c, :] = res.results[c]["out"].astype(np.float32)
    return out, res


# revision 28
# speedup vs baseline: 1.1916x; 1.0745x over previous
"""Trainium2 Bass kernel for nn_Attention_86663850099018.

Math (per batch b, reference semantics):
    xn = x_b / ||x_b rows||                      # (N, E) row-normalized
    S  = xn @ xn.T                               # (N, N) cosine scores, symmetric, in [-1, 1]
    P  = softmax(S, axis=1)                      # row softmax over keys
    U  = P @ h_b                                 # (N, H)
    out = U / frob_norm(U over all batches)      # the reference's H* factor cancels

S is symmetric and bounded so softmax needs no max subtraction, and the
column block of E = exp(S) needed as the stationary operand of the second
matmul equals the row block computed naturally — no transpose of the score
matrix. Rows are relabeled p-major (row = p*16 + t).

Speed design (vs the fp16 serial baseline):
  - Phase A (scores) runs in fp8-e4m3 DoubleRow: the whole E=256
    contraction is one matmul at 0.5 cycles/row.  Rel-err ~2.3e-3
    (tolerance 2e-2).  Phase B (E @ h) stays fp16.
  - exp(S) on ACT is the phase-A wall (~36us); phase A blocks are
    interleaved with phase B column blocks on the PE queue so the PE
    crunches B matmuls while ACT exps the previous A block.
  - Phase B PSUM is drained raw (unnormalized U); row softmax scale and
    the global 1/frobnorm fold into ONE per-row scale at writeback.
    Row sums of exp(S) are per-slab DVE reduces spread through phase B.
  - Sum-of-squares per row via scalar_tensor_tensor accumulate
    (tensor_scalar instruction family; tensor_tensor_reduce faults the HW).
  - Global norm: one 4-byte AllReduce(add); a dummy AllReduce at kernel
    start warms the CC stream (cuts the trigger->start delay ~10x).
    U is pre-scaled by the softmax denominators while the collective is
    in flight; after it lands only a cheap in-place fp16 x(1/gnorm)
    remains.  Output is stored fp16 (well inside tolerance) to halve
    writeback bytes; the host upcasts to float32.

Sharding: data-parallel over batch B=8, one batch per NeuronCore.
"""

import numpy as np

N, B, E, H = 2048, 8, 256, 512
P = 128
NT = N // P      # 16 row tiles
EC = E // P      # 2 contraction chunks (DoubleRow pairs them)
SF = 512         # matmul free-dim chunk
FC = N // SF     # 4 score column chunks
NCORES = 8

WARMUP_CC = True      # dummy AllReduce at start to warm the CC stream
ALLREDUCE = True      # True: AllReduce(add); False: AllGather + local sum
DOUBLEROW = True      # fp8 DoubleRow phase A
_CACHE = {}


def _build():
    import concourse.mybir as mybir
    import concourse.tile as tile
    from concourse import bacc
    from concourse.masks import make_identity

    f32 = mybir.dt.float32
    f16 = mybir.dt.float16
    f8 = mybir.dt.float8e4
    AF = mybir.ActivationFunctionType
    ALU = mybir.AluOpType
    AX = mybir.AxisListType
    DR = mybir.MatmulPerfMode.DoubleRow

    nc = bacc.Bacc("TRN2", target_bir_lowering=False, debug=False, num_devices=NCORES)

    x_d = nc.dram_tensor("x", [N, E], f32, kind="ExternalInput").ap()
    h_d = nc.dram_tensor("h", [N, H], f32, kind="ExternalInput").ap()
    o_d = nc.dram_tensor("out", [N, H], f16, kind="ExternalOutput").ap()

    x_pt = x_d.rearrange("(p t) e -> p t e", t=NT)
    h_pt = h_d.rearrange("(p t) e -> p t e", t=NT)
    o_pt = o_d.rearrange("(p t) e -> p t e", t=NT)

    with tile.TileContext(nc) as tc:
        with (
            tc.tile_pool(name="const", bufs=1) as constp,
            tc.tile_pool(name="eexpp", bufs=1) as eexpp,
            tc.tile_pool(name="hp", bufs=1) as hp,
            tc.tile_pool(name="urawp", bufs=1) as urawp,
            tc.tile_pool(name="zp", bufs=1) as zp,
            tc.tile_pool(name="dramp", bufs=1, space="DRAM") as dramp,
        ):
            if WARMUP_CC:
                w_in = dramp.tile([1, 1], f32)
                w_out = dramp.tile([1, 1], f32)
                wz = zp.tile([1, 1], f32)
                nc.gpsimd.memset(wz[:], 0.0)
                nc.gpsimd.dma_start(w_in[:], wz[:])
                nc.gpsimd.collective_compute(
                    "AllReduce", ALU.add,
                    replica_groups=[list(range(NCORES))],
                    ins=[w_in.opt()], outs=[w_out.opt()],
                )

            ident = constp.tile([P, P], f16)
            make_identity(nc, ident[:])
            ones = constp.tile([P, 1], f32)
            nc.vector.memset(ones[:], 1.0)

            eexp = eexpp.tile([P, NT, N], f16)        # 64 KiB/partition
            h_sb = hp.tile([P, NT, H], f16)           # 16 KiB/partition
            uraw = urawp.tile([P, NT, H], f32)        # 32 KiB/partition
            of16 = urawp.tile([P, NT, H], f16)        # 16 KiB/partition
            xnt_ch = [
                constp.tile([P, EC, SF], f8, name=f"xnt{q}", tag=f"xnt{q}")
                for q in range(FC)
            ]                                          # 4 x 1 KiB/partition

            zpartT = zp.tile([P, FC, NT], f16)
            zsA = zp.tile([P, NT], f32)
            zsB = zp.tile([P, NT], f32)
            zsum = zp.tile([P, NT], f32)
            zinv = zp.tile([P, NT], f32)
            ssqraw = zp.tile([P, NT], f32)
            rs = zp.tile([P, NT], f32)
            t1 = zp.tile([P, NT], f32)
            t2 = zp.tile([P, NT], f32)
            sscol = zp.tile([P, 1], f32)
            sq_scr = zp.tile([P, SF], f32)

            # ---------------- phase 0: load, normalize, transpose -------
            XCH = 8                                    # chunks of 2 row tiles
            with (
                tc.tile_pool(name="xtrans", bufs=1) as xtp,
                tc.tile_pool(name="ph0", bufs=3) as ph0,
                tc.tile_pool(name="psT", bufs=2, space="PSUM") as psT,
            ):
                x_all = xtp.tile([P, NT, E], f32)     # 16 KiB/partition
                htmp = xtp.tile([P, NT, H], f32)      # 32 KiB/partition
                ssq_all = xtp.tile([P, NT], f32)
                lnssq = xtp.tile([P, NT], f32)
                invn = xtp.tile([P, NT], f32)

                xengs = [nc.sync, nc.scalar]
                for ch in range(XCH):
                    t0 = 2 * ch
                    xengs[ch % 2].dma_start(
                        x_all[:, t0 : t0 + 2, :], x_pt[:, t0 : t0 + 2, :]
                    )
                    scr = ph0.tile([P, 2, E], f32, tag="scr")
                    nc.scalar.activation(
                        scr[:], x_all[:, t0 : t0 + 2, :], AF.Square
                    )
                    nc.vector.tensor_reduce(
                        ssq_all[:, t0 : t0 + 2], scr[:], axis=AX.X, op=ALU.add
                    )
                    nc.scalar.activation(
                        lnssq[:, t0 : t0 + 2], ssq_all[:, t0 : t0 + 2], AF.Sqrt
                    )
                    nc.vector.reciprocal(
                        invn[:, t0 : t0 + 2], lnssq[:, t0 : t0 + 2]
                    )
                    for t in (t0, t0 + 1):
                        xn = ph0.tile([P, E], f16, tag="xn")
                        # balance the normalize multiplies across ACT / DVE
                        if t % 2 == 0:
                            nc.scalar.activation(
                                xn[:], x_all[:, t, :], AF.Copy,
                                scale=invn[:, t : t + 1],
                            )
                        else:
                            nc.vector.tensor_scalar_mul(
                                xn[:], x_all[:, t, :], invn[:, t : t + 1]
                            )
                        pt = psT.tile([P, EC, P], f16, tag="pt")
                        for c in range(EC):
                            nc.tensor.transpose(
                                pt[:, c, :], xn[:, c * P : (c + 1) * P], ident[:]
                            )
                        # PSUM->SBUF copy quantizes to fp8
                        nc.vector.tensor_copy(
                            xnt_ch[t // 4][:, :, (t % 4) * P : (t % 4 + 1) * P],
                            pt[:],
                        )

                # h rides the otherwise-idle gpsimd DMA ring; fp16 casts
                # go to DVE in fine chunks so each waits only a small DMA
                for k in range(4):
                    nc.gpsimd.dma_start(
                        htmp[:, 4 * k : 4 * k + 4, :], h_pt[:, 4 * k : 4 * k + 4, :]
                    )
                for k in range(8):
                    nc.vector.tensor_copy(
                        h_sb[:, 2 * k : 2 * k + 2, :], htmp[:, 2 * k : 2 * k + 2, :]
                    )

            # ---------------- phases A+B interleaved ---------------------
            with (
                tc.tile_pool(name="psA", bufs=2, space="PSUM") as psA,
                tc.tile_pool(name="psB", bufs=4, space="PSUM") as psB,
            ):
                def a_block(jc):
                    # 8 matmul pairs + exp straight from PSUM (ACT paces
                    # PSUM recycling; B blocks interleave into the gaps)
                    for ip in range(NT // 2):
                        ps = psA.tile([P, 2, SF], f32, tag="psA")
                        for u in range(2):
                            i = 2 * ip + u
                            if DOUBLEROW:
                                nc.tensor.matmul(
                                    ps[:, u, :],
                                    xnt_ch[i // 4][:, :, (i % 4) * P : (i % 4 + 1) * P],
                                    xnt_ch[jc][:],
                                    start=True, stop=True, perf_mode=DR,
                                )
                            else:
                                for c in range(EC):
                                    nc.tensor.matmul(
                                        ps[:, u, :],
                                        xnt_ch[i // 4][:, c, (i % 4) * P : (i % 4 + 1) * P],
                                        xnt_ch[jc][:, c, :],
                                        start=(c == 0), stop=(c == EC - 1),
                                    )
                        nc.scalar.activation(
                            eexp[:, 2 * ip : 2 * ip + 2, jc * SF : (jc + 1) * SF],
                            ps[:], AF.Exp,
                        )

                def b_block(jcb):
                    # 4 output column blocks; DVE drains U raw, does the
                    # per-slab softmax row sums and the row ssq accumulate
                    for j in range(4 * jcb, 4 * jcb + 4):
                        ps = psB.tile([P, H], f32, tag="psB")
                        for i in range(NT):
                            nc.tensor.matmul(
                                ps[:],
                                eexp[:, i, j * P : (j + 1) * P],
                                h_sb[:, i, :],
                                start=(i == 0), stop=(i == NT - 1),
                            )
                        nc.vector.tensor_copy(uraw[:, j, :], ps[:])
                        nc.vector.scalar_tensor_tensor(
                            out=sq_scr[:],
                            in0=uraw[:, j, :],
                            scalar=1.0,
                            in1=uraw[:, j, :],
                            op0=ALU.mult,
                            op1=ALU.mult,
                            accum_out=ssqraw[:, j : j + 1],
                        )
                        # exp row-sum slabs k = jc*8 + ip, two per j
                        for k in (2 * j, 2 * j + 1):
                            jc, ip = k // 8, k % 8
                            with nc.allow_low_precision(reason="zslab fp16, rel 7e-4"):
                                nc.vector.tensor_reduce(
                                    zpartT[:, jc, 2 * ip : 2 * ip + 2],
                                    eexp[:, 2 * ip : 2 * ip + 2, jc * SF : (jc + 1) * SF],
                                    axis=AX.X,
                                    op=ALU.add,
                                )

                a_block(0)
                a_block(1)
                b_block(0)
                a_block(2)
                b_block(1)
                a_block(3)
                # preload the sqrt/copy ACT table before the tail needs it
                sqpre = zp.tile([1, 1], f32)
                nc.scalar.activation(sqpre[:], ones[:1, :1], AF.Sqrt)
                b_block(2)
                b_block(3)

                # ---------------- tail: global norm + writeback ----------
                nc.vector.scalar_tensor_tensor(
                    out=zsA[:], in0=zpartT[:, 0, :], scalar=0.0,
                    in1=zpartT[:, 1, :], op0=ALU.add, op1=ALU.add,
                )
                nc.vector.scalar_tensor_tensor(
                    out=zsB[:], in0=zpartT[:, 2, :], scalar=0.0,
                    in1=zpartT[:, 3, :], op0=ALU.add, op1=ALU.add,
                )
                nc.vector.scalar_tensor_tensor(
                    out=zsum[:], in0=zsA[:], scalar=0.0,
                    in1=zsB[:], op0=ALU.add, op1=ALU.add,
                )
                nc.vector.reciprocal(zinv[:], zsum[:])
                nc.vector.scalar_tensor_tensor(
                    out=t1[:], in0=ssqraw[:], scalar=1.0,
                    in1=zinv[:], op0=ALU.mult, op1=ALU.mult,
                )
                nc.vector.scalar_tensor_tensor(
                    out=t2[:], in0=t1[:], scalar=1.0,
                    in1=zinv[:], op0=ALU.mult, op1=ALU.mult,
                    accum_out=sscol[:],
                )

                ps1t = psB.tile([P, H], f32, tag="psB")
                ps1 = ps1t[:1, :1]
                nc.tensor.matmul(ps1, ones[:], sscol[:], start=True, stop=True)
                ss11 = zp.tile([1, 1], f32)
                nc.scalar.copy(ss11[:], ps1)

                cc_in = dramp.tile([1, 1], f32)
                gg = zp.tile([1, 1], f32)
                nc.gpsimd.dma_start(cc_in[:], ss11[:])
                if ALLREDUCE:
                    cc_out = dramp.tile([1, 1], f32)
                    nc.gpsimd.collective_compute(
                        "AllReduce", ALU.add,
                        replica_groups=[list(range(NCORES))],
                        ins=[cc_in.opt()], outs=[cc_out.opt()],
                    )
                    nc.sync.dma_start(gg[:], cc_out[:])
                else:
                    cc_out = dramp.tile([NCORES, 1], f32)
                    nc.gpsimd.collective_compute(
                        "AllGather", ALU.bypass,
                        replica_groups=[list(range(NCORES))],
                        ins=[cc_in.opt()], outs=[cc_out.opt()],
                    )
                    agg = zp.tile([NCORES, 1], f32)
                    nc.sync.dma_start(agg[:], cc_out[:])
                    ps2t = psB.tile([P, H], f32, tag="psB")
                    nc.tensor.matmul(
                        ps2t[:1, :1], ones[:NCORES, :], agg[:], start=True, stop=True
                    )
                    nc.scalar.copy(gg[:], ps2t[:1, :1])

                # pre-scale by the row softmax denominators while the
                # collective is in flight (engines are otherwise idle)
                for j in range(NT):
                    if j % 2 == 0:
                        nc.vector.tensor_scalar_mul(
                            of16[:, j, :], uraw[:, j, :], zinv[:, j : j + 1]
                        )
                    else:
                        nc.scalar.activation(
                            of16[:, j, :], uraw[:, j, :], AF.Copy,
                            scale=zinv[:, j : j + 1],
                        )

                lnt = zp.tile([1, 1], f32)
                gsc = zp.tile([1, 1], f32)
                nc.scalar.activation(lnt[:], gg[:], AF.Sqrt)
                nc.vector.reciprocal(gsc[:], lnt[:])
                gbc = zp.tile([P, 1], f32)
                nc.gpsimd.partition_broadcast(gbc[:], gsc[:])

                # after the global norm lands: cheap in-place fp16 scale + DMA
                wengs = [nc.sync, nc.gpsimd, nc.sync, nc.gpsimd]
                for j in range(NT):
                    nc.vector.tensor_scalar_mul(
                        of16[:, j, :], of16[:, j, :], gbc[:]
                    )
                    if j % 2 == 1:
                        wengs[(j // 2) % 4].dma_start(
                            o_pt[:, j - 1 : j + 1, :], of16[:, j - 1 : j + 1, :]
                        )

    nc.compile()
    return nc


def _get_nc():
    if "nc" not in _CACHE:
        _CACHE["nc"] = _build()
    return _CACHE["nc"]


def _in_maps(x, h):
    return [
        {
            "x": np.ascontiguousarray(x[:, c, :]),
            "h": np.ascontiguousarray(h[:, c, :]),
        }
        for c in range(NCORES)
    ]


def kernel(x, h):
    from concourse.bass_utils import run_bass_kernel_spmd

    x = np.asarray(x, dtype=np.float32)
    h = np.asarray(h, dtype=np.float32)
    assert x.shape == (N, B, E) and h.shape == (N, B, H)

    nc = _get_nc()
    res = run_bass_kernel_spmd(nc, _in_maps(x, h), core_ids=list(range(NCORES)))
    out = np.empty((N, B, H), dtype=np.float32)
    for c in range(NCORES):
        out[:, c, :] = res.results[c]["out"].astype(np.float32)
    return out


# Exposed for test.py: run once with tracing to get hardware exec time.
def run_traced(x, h):
    import os
    import shutil

    from concourse.bass_utils import run_bass_kernel_spmd

    x = np.asarray(x, dtype=np.float32)
    h = np.asarray(h, dtype=np.float32)
    nc = _get_nc()
    tdir = "/root/problem/trace_out"
    shutil.rmtree(tdir, ignore_errors=True)
    os.makedirs(tdir, exist_ok=True)
    res = run_bass_kernel_spmd(
        nc, _in_maps(x, h), core_ids=list(range(NCORES)), trace=True, tmpdir=tdir
    )
    out = np.empty((N, B, H), dtype=np.float32)
    for c in range(NCORES):
        out[:, c, :] = res.results[c]["out"].astype(np.float32)
    return out, res
